# revision 2
# baseline (speedup 1.0000x reference)
"""Trainium2 Bass kernel for nn_DiscAdvLossForTarget_min (v8).

Math: loss = (1/B) * sum_b V_b/T_b with a = exp(x - e), w = log1p(a),
V = sum_i a*w, T = sum_i a.

v8 moves the expensive per-row reduction V off the DVE onto the (idle)
tensor engine. Key identity: scaling each row's products by 1/T_b BEFORE
a partition-dim reduction turns the per-row free-axis reduction into a
full sum, which PE can do with a ones-weight matmul:

  loss_core = S * sum_f psum[f],  psum[f] = sum_blocks sum_p pw[p, f]
  pw = a * ww,  ww = (bits(1+a) - K0) * invT   (bit-log log1p at 4x,
                                                invT folded in as the
                                                per-partition scalar2)

Per 128x1000 block:
  DMA   x block (rows n*128..n*128+127 -> fully contiguous 512KB burst)
  DVE   neg_e = -x[:, 1000]                  (61 ns)
  ACT   aa = Exp(x + neg_e), accum_out -> T  (1205 ns; ACT does nothing else)
  DVE   yy = aa + 1 (bf16, 4x)               (321 ns)
  DVE   invT = 1/T col (fp32)                (69 ns)
  DVE   ww = (bits(yy) - K0) * invT (4x)     (321 ns)
  DVE   pw = aa * ww (bf16 tensor_tensor 2x) (581 ns)
  PE    psum[1,500] += ones.T @ pw halves    (2 matmuls, ~420 ns)

Engine busy/core: DMA ~99us (32.8MB at ~332GB/s, the floor), ACT ~77us,
DVE ~87us, PE ~30-50us. DMA-bound with full overlap.

Host: loss = BITLOG_S * (sum of per-class psums over cores) / B.
"""

import numpy as np

import concourse.bacc as bacc
import concourse.bass as bass
import concourse.tile as tile
from concourse import bass_utils, mybir

N_CORES = 8
B_FULL = 65536
C1 = 1001
C = 1000
P = 128
B_SHARD = B_FULL // N_CORES  # 8192
N_BLOCKS = B_SHARD // P  # 64

# bit-log fit: w ~= (bits(y) - K0) * S, a-weighted LS vs log1p
BITLOG_S = 0.00541268
BITLOG_K0 = 16248.447

HALF = 500  # psum bank holds 512 fp32; split the 1000 classes in two

_nc_cache = None


def _build() -> bass.Bass:
    global _nc_cache
    if _nc_cache is not None:
        return _nc_cache

    nc = bacc.Bacc("TRN2", debug=False)
    x = nc.dram_tensor("x", [B_SHARD, C1], mybir.dt.float32, kind="ExternalInput").ap()
    o = nc.dram_tensor("o", [1, C], mybir.dt.float32, kind="ExternalOutput").ap()

    # block n = rows n*128 .. n*128+127: every block DMA is one fully
    # contiguous 512KB HBM range
    x_r = x.rearrange("(n p) m -> p n m", p=P, n=N_BLOCKS)

    with tile.TileContext(nc) as tc:
        with (
            tc.tile_pool(name="xin", bufs=8) as xin,
            tc.tile_pool(name="apool", bufs=4) as apool,
            tc.tile_pool(name="ypool", bufs=3) as ypool,
            tc.tile_pool(name="wpool", bufs=3) as wpool,
            tc.tile_pool(name="pwpool", bufs=4) as pwpool,
            tc.tile_pool(name="nep", bufs=4) as nep,
            tc.tile_pool(name="accp", bufs=1) as accp,
            tc.tile_pool(name="psp", bufs=1, space="PSUM") as psp,
        ):
            T = accp.tile([P, N_BLOCKS], mybir.dt.float32)
            iT = accp.tile([P, N_BLOCKS], mybir.dt.float32)
            ones = accp.tile([P, 1], mybir.dt.bfloat16)
            out_sb = accp.tile([1, C], mybir.dt.float32)
            ps0 = psp.tile([1, HALF], mybir.dt.float32)
            ps1 = psp.tile([1, C - HALF], mybir.dt.float32)
            nc.any.memset(ones, 1.0)

            for n in range(N_BLOCKS):
                xt = xin.tile([P, C1], mybir.dt.float32, tag="xt")
                nc.sync.dma_start(out=xt, in_=x_r[:, n, :])

                neg_e = nep.tile([P, 1], mybir.dt.float32, tag="ne")
                nc.vector.tensor_scalar_mul(neg_e, xt[:, C : C + 1], -1.0)

                aa = apool.tile([P, C], mybir.dt.bfloat16, tag="aa")
                nc.scalar.activation(
                    out=aa,
                    in_=xt[:, 0:C],
                    func=mybir.ActivationFunctionType.Exp,
                    bias=neg_e[:, 0:1],
                    scale=1.0,
                    accum_out=T[:, n : n + 1],
                )

                yy = ypool.tile([P, C], mybir.dt.bfloat16, tag="yy")
                nc.vector.tensor_scalar_add(yy, aa, 1.0)

                nc.vector.reciprocal(iT[:, n : n + 1], T[:, n : n + 1])

                ww = wpool.tile([P, C], mybir.dt.bfloat16, tag="ww")
                nc.vector.tensor_scalar(
                    out=ww,
                    in0=yy.bitcast(mybir.dt.uint16),
                    scalar1=BITLOG_K0,
                    scalar2=iT[:, n : n + 1],
                    op0=mybir.AluOpType.subtract,
                    op1=mybir.AluOpType.mult,
                )

                pw = pwpool.tile([P, C], mybir.dt.bfloat16, tag="pw")
                nc.vector.tensor_tensor(
                    out=pw, in0=aa, in1=ww, op=mybir.AluOpType.mult
                )

                first, last = n == 0, n == N_BLOCKS - 1
                nc.tensor.matmul(
                    ps0, ones, pw[:, 0:HALF], start=first, stop=last
                )
                nc.tensor.matmul(
                    ps1, ones, pw[:, HALF:C], start=first, stop=last
                )

            nc.scalar.copy(out_sb[:, 0:HALF], ps0)
            nc.scalar.copy(out_sb[:, HALF:C], ps1)
            nc.sync.dma_start(out=o, in_=out_sb)

    nc.finalize()
    _nc_cache = nc
    return nc


LAST_RESULTS = None


def kernel(input: np.ndarray, target: np.ndarray | None = None, _trace: bool = False, **_unused) -> np.ndarray:
    global LAST_RESULTS
    input = np.ascontiguousarray(np.asarray(input, dtype=np.float32))
    assert input.shape == (B_FULL, C1), input.shape

    nc = _build()
    in_maps = [
        {"x": input[i * B_SHARD : (i + 1) * B_SHARD]} for i in range(N_CORES)
    ]
    res = bass_utils.run_bass_kernel_spmd(
        nc, in_maps, core_ids=list(range(N_CORES)), trace=_trace
    )
    LAST_RESULTS = res
    total = np.float64(0.0)
    for r in res.results:
        total += np.asarray(r["o"], dtype=np.float64).sum()
    loss = BITLOG_S * total / B_FULL
    return np.float32(loss)
